# revision 1
# baseline (speedup 1.0000x reference)
"""MemTransformerLM (Transformer-XL) forward pass on 8 TRN2 NeuronCores.

Sharding: core c handles batch b = c//2 and tensor-parallel half h = c%2
(heads 8h..8h+8 of 16; FFN inner columns 2048h..2048h+2048 of 4096; vocab
16000h..16000h+16000 for the final logsumexp partials). Pairwise AllReduce
(cores 2b, 2b+1) after the attention output projection and after FFN.

Everything on-chip lives TRANSPOSED: the residual stream is xT[d, i]
([128 part, 8 d-chunks, 512 q]) in bf16, so q/k/v/FFN matmuls consume it
directly as lhsT/rhs with no device-side transposes. Attention scores are
computed transposed too: AC^T[j, i] = k_j . qbw_i with lhsT = kT. The
Transformer-XL rel_shift runs as ONE fused operation: pre[i, j'] is written
to DRAM contiguously and read back through an XBAR transpose-DMA whose
source view has row-stride KLEN-1 — shiftedT[j, i] = pre_flat[i*(KLEN-1)
+ j + QLEN-1] — giving the shifted AND transposed BD in a single DMA.
The softmax denominator comes free from a ones-column appended to V
(PV output row 64 = sum_j prob); probabilities are never normalized —
the 64x512 vec^T is scaled by 1/den instead. LayerNorm in transposed
layout uses matmul-with-ones partition reductions for mean/E[x^2].

All matmuls are bf16 with fp32 PSUM accumulation. Softmax skips
max-subtraction: |scores*scale| < ~8, exp is safe, masked entries are
exactly exp(-1e30*scale) = 0. b2 is passed halved because it is added
on both cores before the AllReduce doubles it.
"""

import numpy as np
import ml_dtypes

import concourse.bass as bass
import concourse.mybir as mybir
import concourse.tile as tile
from concourse import bacc
from concourse.bass_utils import run_bass_kernel_spmd

# Model dims (hardcoded per problem spec)
L = 6
D_MODEL = 1024
D_HEAD = 64
D_INNER = 4096
BSZ = 4
QLEN = 512
MLEN = 512
KLEN = MLEN + QLEN
VOCAB = 32000
SCALE = 1.0 / (D_HEAD ** 0.5)
EPS = 1e-5
NEG = -1e30

NCORES = 8
NDH = 512          # nd per core (8 heads x 64)
DIH = 2048         # ffn inner per core
VSH = VOCAB // 2   # vocab per core (split across the pair)
VT = 400           # vocab tile width
NVT = VSH // VT    # 40

DT = mybir.dt.float32
BF = mybir.dt.bfloat16
F32 = np.float32
BF16 = ml_dtypes.bfloat16

PAIRS = [[0, 1], [2, 3], [4, 5], [6, 7]]

_CACHE: dict = {}


def _build():
    nc = bacc.Bacc("TRN2", target_bir_lowering=False, debug=False, num_devices=NCORES)

    # ---- I/O (all transposed layouts; host preps them) ----
    x0_in = nc.dram_tensor("x0T", [128, 8, QLEN], BF, kind="ExternalInput")
    memT_in = nc.dram_tensor("memT", [L, 128, 8, MLEN], BF, kind="ExternalInput")
    wq_in = nc.dram_tensor("wq", [L, 128, 8, NDH], BF, kind="ExternalInput")
    wk_in = nc.dram_tensor("wk", [L, 128, 8, NDH], BF, kind="ExternalInput")
    wv_in = nc.dram_tensor("wv", [L, 128, 8, NDH], BF, kind="ExternalInput")
    rkT_in = nc.dram_tensor("rkT", [L, 128, 4, KLEN], BF, kind="ExternalInput")
    wo_in = nc.dram_tensor("wo", [L, 128, 4, D_MODEL], BF, kind="ExternalInput")
    w1_in = nc.dram_tensor("w1", [L, 4, 128, 8, 512], BF, kind="ExternalInput")
    w2_in = nc.dram_tensor("w2", [L, 4, 128, 4, D_MODEL], BF, kind="ExternalInput")
    b1_in = nc.dram_tensor("b1", [L, 128, 16], DT, kind="ExternalInput")
    b2_in = nc.dram_tensor("b2T", [L, 128, 8], DT, kind="ExternalInput")
    g1_in = nc.dram_tensor("g1", [L, 128, 8], DT, kind="ExternalInput")
    bg1_in = nc.dram_tensor("bg1", [L, 128, 8], DT, kind="ExternalInput")
    g2_in = nc.dram_tensor("g2", [L, 128, 8], DT, kind="ExternalInput")
    bg2_in = nc.dram_tensor("bg2", [L, 128, 8], DT, kind="ExternalInput")
    bw_in = nc.dram_tensor("bw", [128, 4], DT, kind="ExternalInput")
    br_in = nc.dram_tensor("br", [128, 4], DT, kind="ExternalInput")
    embT_in = nc.dram_tensor("embT", [NVT, 128, 8, VT], BF, kind="ExternalInput")
    ident_in = nc.dram_tensor("ident", [128, 128], BF, kind="ExternalInput")

    xout = nc.dram_tensor("xoutT", [128, 8, QLEN], BF, kind="ExternalOutput")
    lmax_out = nc.dram_tensor("lmax", [128, 4, NVT], DT, kind="ExternalOutput")
    lsum_out = nc.dram_tensor("lsum", [128, 4, NVT], DT, kind="ExternalOutput")

    from contextlib import ExitStack
    with tile.TileContext(nc) as tc:
        with ExitStack() as es:
            constp = es.enter_context(tc.tile_pool(name="const", bufs=1))
            resp = es.enter_context(tc.tile_pool(name="res", bufs=1))
            wtp = es.enter_context(tc.tile_pool(name="wts", bufs=1))
            wsp = es.enter_context(tc.tile_pool(name="wstr", bufs=2))
            actp = es.enter_context(tc.tile_pool(name="act", bufs=1))
            prep = es.enter_context(tc.tile_pool(name="pre2", bufs=2))
            bdp = es.enter_context(tc.tile_pool(name="bd2", bufs=2))
            pbp = es.enter_context(tc.tile_pool(name="pb3", bufs=3))
            evp = es.enter_context(tc.tile_pool(name="ev2", bufs=2))
            lnscr = es.enter_context(tc.tile_pool(name="lnscr", bufs=1))
            hdscr = es.enter_context(tc.tile_pool(name="hdscr", bufs=1))
            psP = es.enter_context(tc.tile_pool(name="ps_proj", bufs=2, space="PSUM"))
            psM = es.enter_context(tc.tile_pool(name="ps_mm", bufs=4, space="PSUM"))
            psV = es.enter_context(tc.tile_pool(name="ps_pv", bufs=2, space="PSUM"))
            dramp = es.enter_context(tc.tile_pool(name="dram", bufs=3, space="DRAM"))
            dramar = es.enter_context(tc.tile_pool(name="dram_ar", bufs=2, space="DRAM"))
            bw_t = constp.tile([128, 4], DT)
            br_t = constp.tile([128, 4], DT)
            ones_t = constp.tile([128, 1], BF)
            eps_t = constp.tile([1, 1], DT)
            ident_t = constp.tile([128, 128], BF)
            nc.sync.dma_start(bw_t[:], bw_in[:])
            nc.sync.dma_start(br_t[:], br_in[:])
            nc.sync.dma_start(ident_t[:], ident_in[:])
            nc.vector.memset(ones_t[:], 1.0)
            nc.vector.memset(eps_t[:], EPS)

            lnp_t = constp.tile([128, 5, L, 8], DT)  # g1|bg1|g2|bg2|b2 blocks
            for i, srct in enumerate((g1_in, bg1_in, g2_in, bg2_in, b2_in)):
                nc.sync.dma_start(lnp_t[:, i, :, :], srct.rearrange("l p c -> p l c"))
            b1_t = constp.tile([128, L, 16], DT)
            nc.sync.dma_start(b1_t[:], b1_in.rearrange("l p c -> p l c"))

            # residual stream, bf16, transposed: xT[p, dc, i] = x[i, 128*dc+p]
            xT = resp.tile([128, 8, QLEN], BF)
            nc.sync.dma_start(xT[:], x0_in[:])
            lmax_sb = resp.tile([128, 4, NVT], DT)
            lsum_sb = resp.tile([128, 4, NVT], DT)

            def ln_stats(l, goff, dcs, mu_ps, e2_ps):
                """Accumulate LN partition-sums for d-chunks dcs."""
                for dc in dcs:
                    nc.tensor.matmul(
                        mu_ps[:], ones_t[:], xT[:, dc, :],
                        start=(dc == 0), stop=(dc == 7),
                    )
                for dc in dcs:
                    sqc = evp.tile([128, QLEN], BF, tag="sq")
                    nc.scalar.square(sqc[:], xT[:, dc, :])
                    nc.tensor.matmul(
                        e2_ps[:], ones_t[:], sqc[:],
                        start=(dc == 0), stop=(dc == 7),
                    )

            def layer_norm(l, goff, mu_ps, e2_ps):
                """Transposed LN over partitions (d); stats already accumulated."""
                pack = lnscr.tile([1, 2 * QLEN], BF, tag="pack")
                muf = lnscr.tile([1, QLEN], DT, tag="muf")
                nc.vector.tensor_scalar_mul(muf[:], mu_ps[:], 1.0 / D_MODEL)
                nc.scalar.copy(pack[:, 0:QLEN], muf[:])
                mu2 = lnscr.tile([1, QLEN], DT, tag="mu2")
                nc.vector.tensor_tensor(mu2[:], muf[:], muf[:],
                                        mybir.AluOpType.mult)
                var = lnscr.tile([1, QLEN], DT, tag="var")
                nc.vector.tensor_scalar_mul(var[:], e2_ps[:], 1.0 / D_MODEL)
                nc.vector.tensor_tensor(var[:], var[:], mu2[:],
                                        mybir.AluOpType.subtract)
                std = lnscr.tile([1, QLEN], DT, tag="std")
                nc.scalar.activation(std[:], var[:],
                                     mybir.ActivationFunctionType.Sqrt,
                                     bias=eps_t[:])
                rstd = lnscr.tile([1, QLEN], DT, tag="rstd")
                nc.vector.reciprocal(rstd[:], std[:])
                nc.scalar.copy(pack[:, QLEN:], rstd[:])
                statb = actp.tile([128, 2 * QLEN], BF, tag="statb")
                nc.gpsimd.partition_broadcast(statb[:], pack[:])
                g = lnp_t[:, 2 * goff, l, :]
                bg = lnp_t[:, 2 * goff + 1, l, :]
                for dc in range(8):
                    xc = xT[:, dc, :]
                    eng = nc.vector if dc % 2 == 0 else nc.gpsimd
                    eng.tensor_tensor(xc, xc, statb[:, 0:QLEN],
                                      mybir.AluOpType.subtract)
                    eng.tensor_tensor(xc, xc, statb[:, QLEN:],
                                      mybir.AluOpType.mult)
                    eng.tensor_scalar(xc, xc, g[:, dc : dc + 1], bg[:, dc : dc + 1],
                                      mybir.AluOpType.mult, mybir.AluOpType.add)

            def ar_send_half(src_sb, hf):
                """DMA half of a [128, 8, 512] partial to DRAM; AllGather it."""
                ar_in = dramar.tile([128, 4, QLEN], BF, tag="arin")
                ar_out = dramar.tile([2, 128, 4, QLEN], BF, tag="arout")
                nc.sync.dma_start(ar_in[:], src_sb[:, 4 * hf : 4 * hf + 4, :])
                nc.gpsimd.collective_compute(
                    "AllGather", mybir.AluOpType.bypass,
                    replica_groups=PAIRS, ins=[ar_in.opt()], outs=[ar_out.opt()],
                )
                return ar_out

            def ar_recv_half(ar_out, hf):
                """Read both AllGather halves (peer accumulated in-flight via
                SDMA CCE add) and add into xT chunks."""
                arr = actp.tile([128, 4, QLEN], BF, tag="asb")
                nc.sync.dma_start(arr[:], ar_out[0])
                nc.gpsimd.dma_start(arr[:], ar_out[1],
                                    accum_op=mybir.AluOpType.add)
                nc.vector.tensor_tensor(xT[:, 4 * hf : 4 * hf + 4, :],
                                        xT[:, 4 * hf : 4 * hf + 4, :],
                                        arr[:], mybir.AluOpType.add)

            for l in range(L):
                # ---- weight + memory loads ----
                wq_t = wtp.tile([128, 8, NDH], BF, tag="wq")
                wk_t = wtp.tile([128, 8, NDH], BF, tag="wk")
                wv_t = wtp.tile([128, 8, NDH], BF, tag="wv")
                rkT_t = wtp.tile([128, 4, KLEN], BF, tag="rk")
                wo_t = wtp.tile([128, 4, D_MODEL], BF, tag="wo")
                nc.sync.dma_start(wq_t[:], wq_in[l])
                nc.sync.dma_start(wk_t[:], wk_in[l])
                nc.sync.dma_start(wv_t[:], wv_in[l])
                nc.sync.dma_start(rkT_t[:], rkT_in[l])
                nc.sync.dma_start(wo_t[:], wo_in[l])
                memT_t = actp.tile([128, 8, MLEN], BF, tag="hT")  # aliases hT slot
                nc.sync.dma_start(memT_t[:], memT_in[l])

                # ---- projections ----
                # q^T (+bw / +br): qb[p, w(0)/r(1), ncc, i]
                qb = actp.tile([128, 2, 4, QLEN], BF, tag="qb")
                for ncc in range(4):
                    nsl = slice(128 * ncc, 128 * ncc + 128)
                    qps = psP.tile([128, QLEN], DT, tag="proj")
                    for dc in range(8):
                        nc.tensor.matmul(
                            qps[:], wq_t[:, dc, nsl], xT[:, dc, :],
                            start=(dc == 0), stop=(dc == 7),
                        )
                    nc.scalar.add(qb[:, 0, ncc, :], qps[:], bw_t[:, ncc : ncc + 1])
                    nc.scalar.add(qb[:, 1, ncc, :], qps[:], br_t[:, ncc : ncc + 1])
                # k^T: kT[p, ncc, j]
                kT = actp.tile([128, 4, KLEN], BF, tag="kT")
                for ncc in range(4):
                    nsl = slice(128 * ncc, 128 * ncc + 128)
                    for kh in range(2):
                        kps = psP.tile([128, QLEN], DT, tag="proj")
                        src = memT_t if kh == 0 else xT
                        for dc in range(8):
                            nc.tensor.matmul(
                                kps[:], wk_t[:, dc, nsl], src[:, dc, :],
                                start=(dc == 0), stop=(dc == 7),
                            )
                        nc.vector.tensor_copy(kT[:, ncc, 512 * kh : 512 * kh + 512],
                                              kps[:])
                # v natural + ones column: vv[p, jc, h, 0:64]=v, [..,64]=1
                vv = actp.tile([128, 8, 8, 66], BF, tag="vv")
                nc.vector.memset(vv[:, :, :, 64:65], 1.0)
                for jc in range(8):
                    vps = psP.tile([128, QLEN], DT, tag="proj")
                    src = memT_t if jc < 4 else xT
                    ksl = slice(128 * (jc % 4), 128 * (jc % 4) + 128)
                    for dc in range(8):
                        nc.tensor.matmul(
                            vps[:], src[:, dc, ksl], wv_t[:, dc, :],
                            start=(dc == 0), stop=(dc == 7),
                        )
                    vsrc = vps.rearrange("p (h d) -> p h d", d=64)
                    if jc % 2 == 0:
                        nc.vector.tensor_copy(vv[:, jc, :, 0:64], vsrc)
                    else:
                        nc.scalar.copy(vv[:, jc, :, 0:64], vsrc)

                # w2 groups reuse the (dead) wq/wk/wv/rk slots during attention
                w2gs = []
                for g, wtag in enumerate(("wq", "wk", "wv", "rk")):
                    w2g = wtp.tile([128, 4, D_MODEL], BF, tag=wtag)
                    nc.sync.dma_start(w2g[:], w2_in[l, g])
                    w2gs.append(w2g)

                pvT_all = actp.tile([128, 4, QLEN], BF, tag="pvT")

                # ---- attention: software-pipelined head pairs ----
                def pre_emit(ncc):
                    """Returns (closures, finish): 16 one-MM closures for the
                    pair's pre matrices, and finish() emitting the DRAM write,
                    fused shift+transpose XBAR reads, and masks."""
                    scr = dramp.tile([2 * QLEN * KLEN], BF, tag="scr")
                    pre_sb = prep.tile([128, 2, 4, KLEN], BF, tag="pre")

                    def mk(hh, ic, jh):
                        B = 64 * hh

                        def f():
                            pps = psM.tile([128, 512], DT, tag="mm")
                            nc.tensor.matmul(
                                pps[:],
                                qb[B : B + 64, 1, ncc, 128 * ic : 128 * ic + 128],
                                rkT_t[B : B + 64, ncc, 512 * jh : 512 * jh + 512],
                                start=True, stop=True,
                            )
                            dst = pre_sb[:, hh, ic, 512 * jh : 512 * jh + 512]
                            if (ic + jh) % 2 == 0:
                                nc.vector.tensor_copy(dst, pps[:])
                            else:
                                nc.scalar.copy(dst, pps[:])
                        return f

                    closures = [mk(hh, ic, jh)
                                for hh in range(2)
                                for ic in range(4) for jh in range(2)]

                    def finish():
                        scr4 = scr.rearrange("(h c p k) -> p h c k", h=2, c=4, p=128, k=KLEN)
                        nc.sync.dma_start(scr4[:], pre_sb[:])
                        bdT = bdp.tile([128, 2, 8, QLEN], BF, tag="bdT")
                        for hh in range(2):
                            for jh in range(2):
                                view = bass.AP(
                                    scr.tensor,
                                    scr.offset + hh * QLEN * KLEN
                                    + (QLEN - 1) + 512 * jh,
                                    [[KLEN - 1, QLEN], [1, 512]],
                                )
                                nc.sync.dma_start_transpose(
                                    bdT[:, hh, 4 * jh : 4 * jh + 4, :], view)
                        for hh in range(2):
                            for jc in range(4, 8):
                                nc.gpsimd.affine_select(
                                    out=bdT[:, hh, jc, :], in_=bdT[:, hh, jc, :],
                                    pattern=[[1, QLEN]],
                                    compare_op=mybir.AluOpType.is_ge,
                                    fill=NEG, base=512 - 128 * jc,
                                    channel_multiplier=-1,
                                )
                        return bdT
                    return closures, finish

                def score_pair(ncc, bdT, fillers):
                    """Score + PV for heads (2ncc, 2ncc+1); BD is added into
                    the AC PSUM by an accumulating identity matmul. The next
                    pair's pre work (fillers) interleaves to cover latency."""
                    pv0 = psV.tile([65, QLEN], DT, tag="pv")
                    pv1 = psV.tile([65, QLEN], DT, tag="pv")
                    pvs = [pv0, pv1]
                    fi = 0
                    for jc in range(8):
                        prbs = []
                        for hh in range(2):
                            B = 64 * hh
                            scp = psM.tile([128, QLEN], DT, tag="mm")
                            nc.tensor.matmul(
                                scp[:],
                                kT[B : B + 64, ncc, 128 * jc : 128 * jc + 128],
                                qb[B : B + 64, 0, ncc, :],
                                start=True, stop=False,
                            )
                            nc.tensor.matmul(
                                scp[:], ident_t[:], bdT[:, hh, jc, :],
                                start=False, stop=True,
                            )
                            prb = pbp.tile([128, QLEN], BF, tag="prb")
                            nc.scalar.activation(prb[:], scp[:],
                                                 mybir.ActivationFunctionType.Exp,
                                                 scale=SCALE)
                            prbs.append(prb)
                        for _ in range(3):
                            if fi < len(fillers):
                                fillers[fi]()
                                fi += 1
                        for hh in range(2):
                            nc.tensor.matmul(
                                pvs[hh][:], vv[:, jc, 2 * ncc + hh, 0:65],
                                prbs[hh][:],
                                start=(jc == 0), stop=(jc == 7),
                            )
                    while fi < len(fillers):
                        fillers[fi]()
                        fi += 1
                    # normalize: vec^T = pv[0:64] / den (row 64), per head
                    for hh in range(2):
                        B = 64 * hh
                        rec = hdscr.tile([1, QLEN], DT, tag="recf")
                        nc.vector.reciprocal(rec[:], pvs[hh][64:65, :])
                        recb = hdscr.tile([64, QLEN], DT, tag="recb")
                        nc.gpsimd.partition_broadcast(recb[:], rec[:])
                        nc.vector.tensor_tensor(pvT_all[B : B + 64, ncc, :],
                                                pvs[hh][0:64, :], recb[:],
                                                mybir.AluOpType.mult)

                cls0, fin0 = pre_emit(0)
                for f in cls0:
                    f()
                bdT_cur = fin0()
                pend = [None]
                for ncc in range(4):
                    if ncc < 3:
                        cls_n, fin_n = pre_emit(ncc + 1)
                        fillers = cls_n + [
                            lambda fn=fin_n: pend.__setitem__(0, fn())]
                    else:
                        fillers = []
                    score_pair(ncc, bdT_cur, fillers)
                    if ncc < 3:
                        bdT_cur = pend[0]

                # ---- attention out projection + halved, overlapped AR ----
                asb = actp.tile([128, 8, QLEN], BF, tag="asb")
                a_ar = []
                for hf in range(2):
                    for dk in range(4 * hf, 4 * hf + 4):
                        ops = psP.tile([128, QLEN], DT, tag="proj")
                        for ncc in range(4):
                            nc.tensor.matmul(
                                ops[:],
                                wo_t[:, ncc, 128 * dk : 128 * dk + 128],
                                pvT_all[:, ncc, :],
                                start=(ncc == 0), stop=(ncc == 3),
                            )
                        if dk % 2 == 0:
                            nc.vector.tensor_copy(asb[:, dk, :], ops[:])
                        else:
                            nc.scalar.copy(asb[:, dk, :], ops[:])
                    a_ar.append(ar_send_half(asb, hf))
                mu_ps = psM.tile([1, QLEN], DT, tag="mm")
                e2_ps = psM.tile([1, QLEN], DT, tag="mm")
                for hf in range(2):
                    ar_recv_half(a_ar[hf], hf)
                    ln_stats(l, 0, range(4 * hf, 4 * hf + 4), mu_ps, e2_ps)
                layer_norm(l, 0, mu_ps, e2_ps)

                # ---- FFN (transposed; w1 streamed in 1MB groups, w2 resident) ----
                hT = actp.tile([128, 16, QLEN], BF, tag="hT")  # aliases memT slot
                for g in range(4):
                    w1g = wsp.tile([128, 8, 512], BF, tag="w1g")
                    nc.sync.dma_start(w1g[:], w1_in[l, g])
                    for i4 in range(4):
                        ic = 4 * g + i4
                        ps = psP.tile([128, QLEN], DT, tag="proj")
                        for dc in range(8):
                            nc.tensor.matmul(
                                ps[:], w1g[:, dc, 128 * i4 : 128 * i4 + 128],
                                xT[:, dc, :],
                                start=(dc == 0), stop=(dc == 7),
                            )
                        nc.scalar.activation(
                            hT[:, ic, :], ps[:], mybir.ActivationFunctionType.Relu,
                            bias=b1_t[:, l, ic : ic + 1],
                        )
                # FFN2: out^T[d, i] = sum_m W2[m, d] hT[m, i]  (+ b2/2)
                fsb = actp.tile([128, 8, QLEN], BF, tag="asb")
                f_ar = []
                for hf in range(2):
                    for dk in range(4 * hf, 4 * hf + 4):
                        fps = psP.tile([128, QLEN], DT, tag="proj")
                        for ic in range(16):
                            nc.tensor.matmul(
                                fps[:],
                                w2gs[ic // 4][:, ic % 4, 128 * dk : 128 * dk + 128],
                                hT[:, ic, :],
                                start=(ic == 0), stop=(ic == 15),
                            )
                        b2c = lnp_t[:, 4, l, dk : dk + 1]
                        if dk % 2 == 0:
                            nc.vector.tensor_scalar(fsb[:, dk, :], fps[:], b2c,
                                                    None, mybir.AluOpType.add)
                        else:
                            nc.scalar.add(fsb[:, dk, :], fps[:], b2c)
                    f_ar.append(ar_send_half(fsb, hf))
                mu2_ps = psM.tile([1, QLEN], DT, tag="mm")
                e22_ps = psM.tile([1, QLEN], DT, tag="mm")
                for hf in range(2):
                    ar_recv_half(f_ar[hf], hf)
                    ln_stats(l, 1, range(4 * hf, 4 * hf + 4), mu2_ps, e22_ps)
                layer_norm(l, 1, mu2_ps, e22_ps)

            # ---- final hidden out + unembed partials ----
            nc.sync.dma_start(xout[:], xT[:])
            for vt in range(NVT):
                et = wsp.tile([128, 8, VT], BF, tag="w1g")
                nc.sync.dma_start(et[:], embT_in[vt])
                for ic in range(4):
                    lps = psM.tile([128, QLEN], DT, tag="mm")
                    for dc in range(8):
                        nc.tensor.matmul(
                            lps[:, 0:VT],
                            xT[:, dc, 128 * ic : 128 * ic + 128],
                            et[:, dc, :],
                            start=(dc == 0), stop=(dc == 7),
                        )
                    nc.vector.tensor_reduce(
                        lmax_sb[:, ic, vt : vt + 1], lps[:, 0:VT],
                        mybir.AxisListType.X, mybir.AluOpType.max,
                    )
                    negm = hdscr.tile([128, 1], DT, tag="negm")
                    nc.vector.tensor_scalar_mul(
                        negm[:], lmax_sb[:, ic, vt : vt + 1], -1.0
                    )
                    lsc = pbp.tile([128, VT], BF, tag="prb")
                    nc.scalar.activation(
                        lsc[:], lps[:, 0:VT], mybir.ActivationFunctionType.Exp,
                        bias=negm[:], accum_out=lsum_sb[:, ic, vt : vt + 1],
                    )
            nc.sync.dma_start(lmax_out[:], lmax_sb[:])
            nc.sync.dma_start(lsum_out[:], lsum_sb[:])

    nc.compile()
    return nc


def _get_nc():
    if "nc" not in _CACHE:
        _CACHE["nc"] = _build()
    return _CACHE["nc"]


def _make_pos():
    pos_seq = np.arange(KLEN - 1, -1, -1, dtype=F32)
    inv_freq = 1.0 / (10000.0 ** (np.arange(0, D_MODEL, 2, dtype=F32) / D_MODEL))
    sin_inp = np.outer(pos_seq, inv_freq).astype(F32)
    return np.concatenate([np.sin(sin_inp), np.cos(sin_inp)], -1).astype(F32)


def _chunkT(a, nch):
    """[L, D, N] -> [L, 128, nch, N]: out[l, p, c, n] = a[l, 128*c+p, n]"""
    L_, D_, N_ = a.shape
    return np.ascontiguousarray(
        a.reshape(L_, nch, 128, N_).transpose(0, 2, 1, 3)).astype(BF16)


def _colsT(a):
    """[L, N] -> [L, 128, N//128] with out[l, p, c] = a[l, 128*c+p]"""
    L_, N_ = a.shape
    return np.ascontiguousarray(
        a.reshape(L_, N_ // 128, 128).transpose(0, 2, 1)).astype(F32)


def _prep_inputs(data, memory, emb, Wq, Wkv, Wr, Wo, ffW1, ffb1, ffW2, ffb2,
                 ln1_g, ln1_b, ln2_g, ln2_b, bias_w, bias_r):
    pos = _make_pos()                                  # [KLEN, D_MODEL]
    rk = np.einsum("kd,ldn->lkn", pos, Wr.astype(F32))  # [L, KLEN, 2*NDH]
    embT = np.ascontiguousarray(emb.T).astype(BF16)    # [D_MODEL, VOCAB]
    bwf = bias_w.reshape(-1).astype(F32)
    brf = bias_r.reshape(-1).astype(F32)

    in_maps = []
    for c in range(NCORES):
        b, h = c // 2, c % 2
        nds = slice(NDH * h, NDH * h + NDH)
        dis = slice(DIH * h, DIH * h + DIH)
        # rkT[l, p, ncc, j'] = rk[l, j', nds + 128*ncc + p]
        rkTh = np.ascontiguousarray(
            rk[:, :, nds].transpose(0, 2, 1).reshape(L, 4, 128, KLEN)
            .transpose(0, 2, 1, 3)).astype(BF16)
        # memT[l, p, dc, j] = memory[l, b, j, 128*dc + p]
        memTb = _chunkT(
            np.ascontiguousarray(memory[:, b].transpose(0, 2, 1)), 8)
        x0 = emb[np.asarray(data[b])].astype(F32)      # [512, 1024]
        x0T = np.ascontiguousarray(
            x0.T.reshape(8, 128, QLEN).transpose(1, 0, 2)).astype(BF16)
        # w1 grouped: [L, 4, 128, 8, 512]; w1g[l,g,p,dc,c] = W1[l, 128dc+p, dis+512g+c]
        A = np.asarray(ffW1[:, :, dis], dtype=F32)      # [L, 1024, 2048]
        w1g = np.ascontiguousarray(
            A.reshape(L, 8, 128, 4, 512).transpose(0, 3, 2, 1, 4)).astype(BF16)
        # w2 grouped: [L, 4, 128, 4, 1024];
        # w2[l,g,p,i4,d] = W2[l, dis + 128*(4g+i4) + p, d]
        Bm = np.asarray(ffW2[:, dis, :], dtype=F32)     # [L, 2048, 1024]
        w2c = np.ascontiguousarray(
            Bm.reshape(L, 4, 4, 128, D_MODEL).transpose(0, 1, 3, 2, 4)).astype(BF16)
        embTh = embT[:, VSH * h : VSH * h + VSH]
        embT4 = np.ascontiguousarray(
            embTh.reshape(8, 128, NVT, VT).transpose(2, 1, 0, 3))
        in_maps.append({
            "x0T": x0T,
            "memT": memTb,
            "wq": _chunkT(np.asarray(Wq[:, :, nds], dtype=F32), 8),
            "wk": _chunkT(np.asarray(Wkv[:, :, nds], dtype=F32), 8),
            "wv": _chunkT(np.asarray(
                Wkv[:, :, D_MODEL + NDH * h : D_MODEL + NDH * h + NDH],
                dtype=F32), 8),
            "rkT": rkTh,
            "wo": _chunkT(np.asarray(Wo[:, nds, :], dtype=F32), 4),
            "w1": w1g,
            "w2": w2c,
            "b1": np.ascontiguousarray(
                np.asarray(ffb1[:, dis], dtype=F32)
                .reshape(L, 16, 128).transpose(0, 2, 1)),
            "b2T": _colsT(np.asarray(ffb2, dtype=F32) * 0.5),  # AR doubles it
            "g1": _colsT(np.asarray(ln1_g, dtype=F32)),
            "bg1": _colsT(np.asarray(ln1_b, dtype=F32)),
            "g2": _colsT(np.asarray(ln2_g, dtype=F32)),
            "bg2": _colsT(np.asarray(ln2_b, dtype=F32)),
            "bw": np.ascontiguousarray(bwf[nds].reshape(4, 128).T),
            "br": np.ascontiguousarray(brf[nds].reshape(4, 128).T),
            "embT": embT4,
            "ident": np.eye(128, dtype=F32).astype(BF16),
        })
    return in_maps


def _combine(results, target, emb):
    nll = np.zeros((BSZ, QLEN), dtype=np.float64)
    for b in range(BSZ):
        r0, r1 = results[2 * b], results[2 * b + 1]
        lm = np.concatenate([r0["lmax"], r1["lmax"]], axis=-1).astype(np.float64)
        ls = np.concatenate([r0["lsum"], r1["lsum"]], axis=-1).astype(np.float64)
        M = lm.max(-1)                                   # [128, 4]
        Z = (ls * np.exp(lm - M[..., None])).sum(-1)     # [128, 4]
        logZ = (M + np.log(Z)).transpose(1, 0).reshape(QLEN)  # i = 128*ic + p
        # xoutT[p, dc, i] = x[i, 128*dc+p]
        xTb = r0["xoutT"].astype(np.float64)             # [128, 8, 512]
        xf = xTb.transpose(2, 1, 0).reshape(QLEN, D_MODEL)
        et = emb[np.asarray(target[b])].astype(BF16).astype(np.float64)
        tgt = (xf * et).sum(-1)
        nll[b] = logZ - tgt
    return nll.astype(F32).reshape(-1).reshape(QLEN, BSZ)


def kernel(**inputs):
    nc = _get_nc()
    data = np.asarray(inputs["data"])
    target = np.asarray(inputs["target"])
    emb = np.asarray(inputs["emb"], dtype=F32)
    in_maps = _prep_inputs(
        data, np.asarray(inputs["memory"], dtype=F32), emb,
        np.asarray(inputs["Wq"], dtype=F32), np.asarray(inputs["Wkv"], dtype=F32),
        np.asarray(inputs["Wr"], dtype=F32), np.asarray(inputs["Wo"], dtype=F32),
        np.asarray(inputs["ffW1"], dtype=F32), np.asarray(inputs["ffb1"], dtype=F32),
        np.asarray(inputs["ffW2"], dtype=F32), np.asarray(inputs["ffb2"], dtype=F32),
        np.asarray(inputs["ln1_g"], dtype=F32), np.asarray(inputs["ln1_b"], dtype=F32),
        np.asarray(inputs["ln2_g"], dtype=F32), np.asarray(inputs["ln2_b"], dtype=F32),
        np.asarray(inputs["bias_w"], dtype=F32), np.asarray(inputs["bias_r"], dtype=F32),
    )
    res = run_bass_kernel_spmd(nc, in_maps, core_ids=list(range(NCORES)))
    return _combine(res.results, target, emb)



# revision 10
# speedup vs baseline: 1.0114x; 1.0114x over previous
"""MemTransformerLM (Transformer-XL) forward pass on 8 TRN2 NeuronCores.

Sharding: core c handles batch b = c//2 and tensor-parallel half h = c%2
(heads 8h..8h+8 of 16; FFN inner columns 2048h..2048h+2048 of 4096; vocab
16000h..16000h+16000 for the final logsumexp partials). Pairwise AllReduce
(cores 2b, 2b+1) after the attention output projection and after FFN.

Everything on-chip lives TRANSPOSED: the residual stream is xT[d, i]
([128 part, 8 d-chunks, 512 q]) in bf16, so q/k/v/FFN matmuls consume it
directly as lhsT/rhs with no device-side transposes. Attention scores are
computed transposed too: AC^T[j, i] = k_j . qbw_i with lhsT = kT. The
Transformer-XL rel_shift runs as ONE fused operation: pre[i, j'] is written
to DRAM contiguously and read back through an XBAR transpose-DMA whose
source view has row-stride KLEN-1 — shiftedT[j, i] = pre_flat[i*(KLEN-1)
+ j + QLEN-1] — giving the shifted AND transposed BD in a single DMA.
The softmax denominator comes free from a ones-column appended to V
(PV output row 64 = sum_j prob); probabilities are never normalized —
the 64x512 vec^T is scaled by 1/den instead. LayerNorm in transposed
layout uses matmul-with-ones partition reductions for mean/E[x^2].

All matmuls are bf16 with fp32 PSUM accumulation. Softmax skips
max-subtraction: |scores*scale| < ~8, exp is safe, masked entries are
exactly exp(-1e30*scale) = 0. b2 is passed halved because it is added
on both cores before the AllReduce doubles it.
"""

import numpy as np
import ml_dtypes

import concourse.bass as bass
import concourse.mybir as mybir
import concourse.tile as tile
from concourse import bacc
from concourse.bass_utils import run_bass_kernel_spmd
from concourse.tile_rust import add_dep_helper

# Model dims (hardcoded per problem spec)
L = 6
D_MODEL = 1024
D_HEAD = 64
D_INNER = 4096
BSZ = 4
QLEN = 512
MLEN = 512
KLEN = MLEN + QLEN
VOCAB = 32000
SCALE = 1.0 / (D_HEAD ** 0.5)
EPS = 1e-5
NEG = -1e30

NCORES = 8
NDH = 512          # nd per core (8 heads x 64)
DIH = 2048         # ffn inner per core
VSH = VOCAB // 2   # vocab per core (split across the pair)
VT = 400           # vocab tile width
NVT = VSH // VT    # 40

DT = mybir.dt.float32
BF = mybir.dt.bfloat16
F32 = np.float32
BF16 = ml_dtypes.bfloat16

PAIRS = [[0, 1], [2, 3], [4, 5], [6, 7]]

_CACHE: dict = {}


def _build():
    nc = bacc.Bacc("TRN2", target_bir_lowering=False, debug=False, num_devices=NCORES)

    # ---- I/O (all transposed layouts; host preps them) ----
    x0_in = nc.dram_tensor("x0T", [128, 8, QLEN], BF, kind="ExternalInput")
    memT_in = nc.dram_tensor("memT", [L, 128, 8, MLEN], BF, kind="ExternalInput")
    wq_in = nc.dram_tensor("wq", [L, 128, 8, NDH], BF, kind="ExternalInput")
    wk_in = nc.dram_tensor("wk", [L, 128, 8, NDH], BF, kind="ExternalInput")
    wv_in = nc.dram_tensor("wv", [L, 128, 8, NDH], BF, kind="ExternalInput")
    rkT_in = nc.dram_tensor("rkT", [L, 128, 4, KLEN], BF, kind="ExternalInput")
    wo_in = nc.dram_tensor("wo", [L, 128, 4, D_MODEL], BF, kind="ExternalInput")
    w1_in = nc.dram_tensor("w1", [L, 4, 128, 8, 512], BF, kind="ExternalInput")
    w2_in = nc.dram_tensor("w2", [L, 4, 128, 4, D_MODEL], BF, kind="ExternalInput")
    b1_in = nc.dram_tensor("b1", [L, 128, 16], DT, kind="ExternalInput")
    b2_in = nc.dram_tensor("b2T", [L, 128, 8], DT, kind="ExternalInput")
    g1_in = nc.dram_tensor("g1", [L, 128, 8], DT, kind="ExternalInput")
    bg1_in = nc.dram_tensor("bg1", [L, 128, 8], DT, kind="ExternalInput")
    g2_in = nc.dram_tensor("g2", [L, 128, 8], DT, kind="ExternalInput")
    bg2_in = nc.dram_tensor("bg2", [L, 128, 8], DT, kind="ExternalInput")
    bw_in = nc.dram_tensor("bw", [128, 4], DT, kind="ExternalInput")
    br_in = nc.dram_tensor("br", [128, 4], DT, kind="ExternalInput")
    embT_in = nc.dram_tensor("embT", [NVT, 128, 8, VT], BF, kind="ExternalInput")
    ident_in = nc.dram_tensor("ident", [128, 128], BF, kind="ExternalInput")

    xout = nc.dram_tensor("xoutT", [128, 8, QLEN], BF, kind="ExternalOutput")
    lmax_out = nc.dram_tensor("lmax", [128, 4, NVT], DT, kind="ExternalOutput")
    lsum_out = nc.dram_tensor("lsum", [128, 4, NVT], DT, kind="ExternalOutput")

    # Pair-shared HBM exchange buffers: on TRN2/LNC1 addr_space="Shared"
    # DRAM lives in the HBM domain shared between cores (2k, 2k+1), so a
    # plain local DMA publishes a partial to the TP peer; a tiny AllGather
    # acts as the arrival barrier. Slot 0 = attention AR, slot 1 = FFN AR;
    # layer l+1's reuse of a slot is safe because the peer's read of
    # exchange k-2 strictly precedes its barrier trigger for k-1.
    shx = [
        nc.dram_tensor(f"shx{i}", [2, 128, 8, QLEN], BF,
                       kind="Internal", addr_space="Shared")
        for i in range(2)
    ]

    from contextlib import ExitStack
    with tile.TileContext(nc) as tc:
        with ExitStack() as es:
            constp = es.enter_context(tc.tile_pool(name="const", bufs=1))
            resp = es.enter_context(tc.tile_pool(name="res", bufs=1))
            wtp = es.enter_context(tc.tile_pool(name="wts", bufs=1))
            wsp = es.enter_context(tc.tile_pool(name="wstr", bufs=2))
            actp = es.enter_context(tc.tile_pool(name="act", bufs=1))
            prep = es.enter_context(tc.tile_pool(name="pre2", bufs=2))
            bdp = es.enter_context(tc.tile_pool(name="bd2", bufs=2))
            pbp = es.enter_context(tc.tile_pool(name="pb3", bufs=3))
            evp = es.enter_context(tc.tile_pool(name="ev2", bufs=2))
            lnscr = es.enter_context(tc.tile_pool(name="lnscr", bufs=1))
            hdscr = es.enter_context(tc.tile_pool(name="hdscr", bufs=1))
            psP = es.enter_context(tc.tile_pool(name="ps_proj", bufs=2, space="PSUM"))
            psM = es.enter_context(tc.tile_pool(name="ps_mm", bufs=4, space="PSUM"))
            psV = es.enter_context(tc.tile_pool(name="ps_pv", bufs=2, space="PSUM"))
            dramp = es.enter_context(tc.tile_pool(name="dram", bufs=3, space="DRAM"))
            dramar = es.enter_context(tc.tile_pool(name="dram_ar", bufs=2, space="DRAM"))
            bw_t = constp.tile([128, 4], DT)
            br_t = constp.tile([128, 4], DT)
            ones_t = constp.tile([128, 1], BF)
            eps_t = constp.tile([1, 1], DT)
            ident_t = constp.tile([128, 128], BF)
            nc.sync.dma_start(bw_t[:], bw_in[:])
            nc.sync.dma_start(br_t[:], br_in[:])
            nc.sync.dma_start(ident_t[:], ident_in[:])
            nc.vector.memset(ones_t[:], 1.0)
            nc.vector.memset(eps_t[:], EPS)

            lnp_t = constp.tile([128, 5, L, 8], DT)  # g1|bg1|g2|bg2|b2 blocks
            for i, srct in enumerate((g1_in, bg1_in, g2_in, bg2_in, b2_in)):
                nc.sync.dma_start(lnp_t[:, i, :, :], srct.rearrange("l p c -> p l c"))
            b1_t = constp.tile([128, L, 16], DT)
            nc.sync.dma_start(b1_t[:], b1_in.rearrange("l p c -> p l c"))

            # residual stream, bf16, transposed: xT[p, dc, i] = x[i, 128*dc+p]
            xT = resp.tile([128, 8, QLEN], BF)
            nc.sync.dma_start(xT[:], x0_in[:])
            lmax_sb = resp.tile([128, 4, NVT], DT)
            lsum_sb = resp.tile([128, 4, NVT], DT)

            def ln_stats(l, goff, dcs, mu_ps, e2_ps):
                """Accumulate LN partition-sums for d-chunks dcs."""
                for dc in dcs:
                    nc.tensor.matmul(
                        mu_ps[:], ones_t[:], xT[:, dc, :],
                        start=(dc == 0), stop=(dc == 7),
                    )
                for dc in dcs:
                    sqc = evp.tile([128, QLEN], BF, tag="sq")
                    nc.scalar.square(sqc[:], xT[:, dc, :])
                    nc.tensor.matmul(
                        e2_ps[:], ones_t[:], sqc[:],
                        start=(dc == 0), stop=(dc == 7),
                    )

            def layer_norm(l, goff, mu_ps, e2_ps):
                """Transposed LN over partitions (d); stats already accumulated."""
                pack = lnscr.tile([1, 2 * QLEN], BF, tag="pack")
                muf = lnscr.tile([1, QLEN], DT, tag="muf")
                nc.vector.tensor_scalar_mul(muf[:], mu_ps[:], 1.0 / D_MODEL)
                nc.scalar.copy(pack[:, 0:QLEN], muf[:])
                mu2 = lnscr.tile([1, QLEN], DT, tag="mu2")
                nc.vector.tensor_tensor(mu2[:], muf[:], muf[:],
                                        mybir.AluOpType.mult)
                var = lnscr.tile([1, QLEN], DT, tag="var")
                nc.vector.tensor_scalar_mul(var[:], e2_ps[:], 1.0 / D_MODEL)
                nc.vector.tensor_tensor(var[:], var[:], mu2[:],
                                        mybir.AluOpType.subtract)
                std = lnscr.tile([1, QLEN], DT, tag="std")
                nc.scalar.activation(std[:], var[:],
                                     mybir.ActivationFunctionType.Sqrt,
                                     bias=eps_t[:])
                rstd = lnscr.tile([1, QLEN], DT, tag="rstd")
                nc.vector.reciprocal(rstd[:], std[:])
                nc.scalar.copy(pack[:, QLEN:], rstd[:])
                statb = actp.tile([128, 2 * QLEN], BF, tag="statb")
                nc.gpsimd.partition_broadcast(statb[:], pack[:])
                g = lnp_t[:, 2 * goff, l, :]
                bg = lnp_t[:, 2 * goff + 1, l, :]
                for dc in range(8):
                    xc = xT[:, dc, :]
                    eng = nc.vector if dc % 2 == 0 else nc.gpsimd
                    eng.tensor_tensor(xc, xc, statb[:, 0:QLEN],
                                      mybir.AluOpType.subtract)
                    eng.tensor_tensor(xc, xc, statb[:, QLEN:],
                                      mybir.AluOpType.mult)
                    eng.tensor_scalar(xc, xc, g[:, dc : dc + 1], bg[:, dc : dc + 1],
                                      mybir.AluOpType.mult, mybir.AluOpType.add)

            par_sync = nc.sync.partition_id() % 2
            par_sync_inv = 1 - par_sync
            par_scal = nc.scalar.partition_id() % 2
            par_scal_inv = 1 - par_scal

            def xch_send_half(src_sb, hf, slot):
                """Publish half of my [128, 8, 512] partial to the pair-shared
                HBM slot (my parity lane)."""
                return nc.sync.dma_start(
                    shx[slot][par_sync, :, 4 * hf : 4 * hf + 4, :],
                    src_sb[:, 4 * hf : 4 * hf + 4, :])

            def xch_barrier(writes):
                """Tiny pair AllGather; completion proves both peers' shared
                writes (deps below) have landed."""
                bin_t = dramar.tile([1, 1], mybir.dt.uint8, tag="arin")
                bout_t = dramar.tile([2, 1, 1], mybir.dt.uint8, tag="arout")
                cc = nc.gpsimd.collective_compute(
                    "AllGather", mybir.AluOpType.bypass,
                    replica_groups=PAIRS, ins=[bin_t.opt()], outs=[bout_t.opt()],
                )
                for w in writes:
                    add_dep_helper(cc.ins, w.ins,
                                   reason="barrier after shared writes")
                return cc

            def xch_local_add(src_sb, hf):
                """xT += my own partial half (independent of the peer)."""
                nc.vector.tensor_tensor(xT[:, 4 * hf : 4 * hf + 4, :],
                                        xT[:, 4 * hf : 4 * hf + 4, :],
                                        src_sb[:, 4 * hf : 4 * hf + 4, :],
                                        mybir.AluOpType.add)

            def xch_recv_half(hf, slot, cc):
                """Read the peer's half from shared HBM and add into xT."""
                arr = actp.tile([128, 4, QLEN], BF, tag="asb")
                rd = nc.scalar.dma_start(
                    arr[:], shx[slot][par_scal_inv, :, 4 * hf : 4 * hf + 4, :])
                add_dep_helper(rd.ins, cc.ins, reason="read after barrier")
                nc.gpsimd.tensor_tensor(xT[:, 4 * hf : 4 * hf + 4, :],
                                        xT[:, 4 * hf : 4 * hf + 4, :],
                                        arr[:], mybir.AluOpType.add)

            for l in range(L):
                # ---- weight + memory loads ----
                wq_t = wtp.tile([128, 8, NDH], BF, tag="wq")
                wk_t = wtp.tile([128, 8, NDH], BF, tag="wk")
                wv_t = wtp.tile([128, 8, NDH], BF, tag="wv")
                rkT_t = wtp.tile([128, 4, KLEN], BF, tag="rk")
                wo_t = wtp.tile([128, 4, D_MODEL], BF, tag="wo")
                nc.sync.dma_start(wq_t[:], wq_in[l])
                nc.sync.dma_start(wk_t[:], wk_in[l])
                nc.sync.dma_start(wv_t[:], wv_in[l])
                nc.sync.dma_start(rkT_t[:], rkT_in[l])
                nc.sync.dma_start(wo_t[:], wo_in[l])
                memT_t = actp.tile([128, 8, MLEN], BF, tag="hT")  # aliases hT slot
                nc.sync.dma_start(memT_t[:], memT_in[l])

                # ---- projections ----
                # q^T (+bw / +br): qb[p, w(0)/r(1), ncc, i]
                qb = actp.tile([128, 2, 4, QLEN], BF, tag="qb")
                for ncc in range(4):
                    nsl = slice(128 * ncc, 128 * ncc + 128)
                    qps = psP.tile([128, QLEN], DT, tag="proj")
                    for dc in range(8):
                        nc.tensor.matmul(
                            qps[:], wq_t[:, dc, nsl], xT[:, dc, :],
                            start=(dc == 0), stop=(dc == 7),
                        )
                    nc.scalar.add(qb[:, 0, ncc, :], qps[:], bw_t[:, ncc : ncc + 1])
                    nc.scalar.add(qb[:, 1, ncc, :], qps[:], br_t[:, ncc : ncc + 1])
                # k^T: kT[p, ncc, j]
                kT = actp.tile([128, 4, KLEN], BF, tag="kT")
                for ncc in range(4):
                    nsl = slice(128 * ncc, 128 * ncc + 128)
                    for kh in range(2):
                        kps = psP.tile([128, QLEN], DT, tag="proj")
                        src = memT_t if kh == 0 else xT
                        for dc in range(8):
                            nc.tensor.matmul(
                                kps[:], wk_t[:, dc, nsl], src[:, dc, :],
                                start=(dc == 0), stop=(dc == 7),
                            )
                        nc.vector.tensor_copy(kT[:, ncc, 512 * kh : 512 * kh + 512],
                                              kps[:])
                # v natural + ones column: vv[p, jc, h, 0:64]=v, [..,64]=1
                vv = actp.tile([128, 8, 8, 66], BF, tag="vv")
                nc.vector.memset(vv[:, :, :, 64:65], 1.0)
                for jc in range(8):
                    vps = psP.tile([128, QLEN], DT, tag="proj")
                    src = memT_t if jc < 4 else xT
                    ksl = slice(128 * (jc % 4), 128 * (jc % 4) + 128)
                    for dc in range(8):
                        nc.tensor.matmul(
                            vps[:], src[:, dc, ksl], wv_t[:, dc, :],
                            start=(dc == 0), stop=(dc == 7),
                        )
                    vsrc = vps.rearrange("p (h d) -> p h d", d=64)
                    if jc % 2 == 0:
                        nc.vector.tensor_copy(vv[:, jc, :, 0:64], vsrc)
                    else:
                        nc.scalar.copy(vv[:, jc, :, 0:64], vsrc)

                # w2 groups reuse the (dead) wq/wk/wv/rk slots during attention
                w2gs = []
                for g, wtag in enumerate(("wq", "wk", "wv", "rk")):
                    w2g = wtp.tile([128, 4, D_MODEL], BF, tag=wtag)
                    nc.sync.dma_start(w2g[:], w2_in[l, g])
                    w2gs.append(w2g)

                pvT_all = actp.tile([128, 4, QLEN], BF, tag="pvT")

                # ---- attention: software-pipelined head pairs ----
                def pre_emit(ncc):
                    """Returns (closures, finish): 16 one-MM closures for the
                    pair's pre matrices, and finish() emitting the DRAM write,
                    fused shift+transpose XBAR reads, and masks."""
                    scr = dramp.tile([2 * QLEN * KLEN], BF, tag="scr")
                    pre_sb = prep.tile([128, 2, 4, KLEN], BF, tag="pre")

                    def mk(hh, ic, jh):
                        B = 64 * hh

                        def f():
                            pps = psM.tile([128, 512], DT, tag="mm")
                            nc.tensor.matmul(
                                pps[:],
                                qb[B : B + 64, 1, ncc, 128 * ic : 128 * ic + 128],
                                rkT_t[B : B + 64, ncc, 512 * jh : 512 * jh + 512],
                                start=True, stop=True,
                            )
                            dst = pre_sb[:, hh, ic, 512 * jh : 512 * jh + 512]
                            if (ic + jh) % 2 == 0:
                                nc.vector.tensor_copy(dst, pps[:])
                            else:
                                nc.scalar.copy(dst, pps[:])
                        return f

                    closures = [mk(hh, ic, jh)
                                for hh in range(2)
                                for ic in range(4) for jh in range(2)]

                    def finish():
                        scr4 = scr.rearrange("(h c p k) -> p h c k", h=2, c=4, p=128, k=KLEN)
                        nc.sync.dma_start(scr4[:], pre_sb[:])
                        bdT = bdp.tile([128, 2, 8, QLEN], BF, tag="bdT")
                        for hh in range(2):
                            for jh in range(2):
                                view = bass.AP(
                                    scr.tensor,
                                    scr.offset + hh * QLEN * KLEN
                                    + (QLEN - 1) + 512 * jh,
                                    [[KLEN - 1, QLEN], [1, 512]],
                                )
                                nc.sync.dma_start_transpose(
                                    bdT[:, hh, 4 * jh : 4 * jh + 4, :], view)
                        for hh in range(2):
                            for jc in range(4, 8):
                                nc.gpsimd.affine_select(
                                    out=bdT[:, hh, jc, :], in_=bdT[:, hh, jc, :],
                                    pattern=[[1, QLEN]],
                                    compare_op=mybir.AluOpType.is_ge,
                                    fill=NEG, base=512 - 128 * jc,
                                    channel_multiplier=-1,
                                )
                        return bdT
                    return closures, finish

                def score_pair(ncc, bdT, fillers):
                    """Score + PV for heads (2ncc, 2ncc+1); BD is added into
                    the AC PSUM by an accumulating identity matmul. The next
                    pair's pre work (fillers) interleaves to cover latency."""
                    pv0 = psV.tile([65, QLEN], DT, tag="pv")
                    pv1 = psV.tile([65, QLEN], DT, tag="pv")
                    pvs = [pv0, pv1]
                    fi = 0
                    for jc in range(8):
                        prbs = []
                        for hh in range(2):
                            B = 64 * hh
                            scp = psM.tile([128, QLEN], DT, tag="mm")
                            nc.tensor.matmul(
                                scp[:],
                                kT[B : B + 64, ncc, 128 * jc : 128 * jc + 128],
                                qb[B : B + 64, 0, ncc, :],
                                start=True, stop=False,
                            )
                            nc.tensor.matmul(
                                scp[:], ident_t[:], bdT[:, hh, jc, :],
                                start=False, stop=True,
                            )
                            prb = pbp.tile([128, QLEN], BF, tag="prb")
                            nc.scalar.activation(prb[:], scp[:],
                                                 mybir.ActivationFunctionType.Exp,
                                                 scale=SCALE)
                            prbs.append(prb)
                        for _ in range(3):
                            if fi < len(fillers):
                                fillers[fi]()
                                fi += 1
                        for hh in range(2):
                            nc.tensor.matmul(
                                pvs[hh][:], vv[:, jc, 2 * ncc + hh, 0:65],
                                prbs[hh][:],
                                start=(jc == 0), stop=(jc == 7),
                            )
                    while fi < len(fillers):
                        fillers[fi]()
                        fi += 1
                    # normalize: vec^T = pv[0:64] / den (row 64), per head
                    for hh in range(2):
                        B = 64 * hh
                        rec = hdscr.tile([1, QLEN], DT, tag="recf")
                        nc.vector.reciprocal(rec[:], pvs[hh][64:65, :])
                        recb = hdscr.tile([64, QLEN], DT, tag="recb")
                        nc.gpsimd.partition_broadcast(recb[:], rec[:])
                        nc.vector.tensor_tensor(pvT_all[B : B + 64, ncc, :],
                                                pvs[hh][0:64, :], recb[:],
                                                mybir.AluOpType.mult)

                cls0, fin0 = pre_emit(0)
                for f in cls0:
                    f()
                bdT_cur = fin0()
                pend = [None]
                for ncc in range(4):
                    if ncc < 3:
                        cls_n, fin_n = pre_emit(ncc + 1)
                        fillers = cls_n + [
                            lambda fn=fin_n: pend.__setitem__(0, fn())]
                    else:
                        fillers = []
                    score_pair(ncc, bdT_cur, fillers)
                    if ncc < 3:
                        bdT_cur = pend[0]

                # ---- attention out projection + halved, overlapped AR ----
                asb = actp.tile([128, 8, QLEN], BF, tag="asb")
                a_ar = []
                for hf in range(2):
                    for dk in range(4 * hf, 4 * hf + 4):
                        ops = psP.tile([128, QLEN], DT, tag="proj")
                        for ncc in range(4):
                            nc.tensor.matmul(
                                ops[:],
                                wo_t[:, ncc, 128 * dk : 128 * dk + 128],
                                pvT_all[:, ncc, :],
                                start=(ncc == 0), stop=(ncc == 3),
                            )
                        if dk % 2 == 0:
                            nc.vector.tensor_copy(asb[:, dk, :], ops[:])
                        else:
                            nc.scalar.copy(asb[:, dk, :], ops[:])
                    a_ar.append(xch_send_half(asb, hf, 0))
                    xch_local_add(asb, hf)
                a_cc = xch_barrier(a_ar)
                mu_ps = psM.tile([1, QLEN], DT, tag="mm")
                e2_ps = psM.tile([1, QLEN], DT, tag="mm")
                for hf in range(2):
                    xch_recv_half(hf, 0, a_cc)
                    ln_stats(l, 0, range(4 * hf, 4 * hf + 4), mu_ps, e2_ps)
                layer_norm(l, 0, mu_ps, e2_ps)

                # ---- FFN (transposed; w1 streamed in 1MB groups, w2 resident) ----
                hT = actp.tile([128, 16, QLEN], BF, tag="hT")  # aliases memT slot
                for g in range(4):
                    w1g = wsp.tile([128, 8, 512], BF, tag="w1g")
                    nc.sync.dma_start(w1g[:], w1_in[l, g])
                    for i4 in range(4):
                        ic = 4 * g + i4
                        ps = psP.tile([128, QLEN], DT, tag="proj")
                        for dc in range(8):
                            nc.tensor.matmul(
                                ps[:], w1g[:, dc, 128 * i4 : 128 * i4 + 128],
                                xT[:, dc, :],
                                start=(dc == 0), stop=(dc == 7),
                            )
                        nc.scalar.activation(
                            hT[:, ic, :], ps[:], mybir.ActivationFunctionType.Relu,
                            bias=b1_t[:, l, ic : ic + 1],
                        )
                # FFN2: out^T[d, i] = sum_m W2[m, d] hT[m, i]  (+ b2/2)
                fsb = actp.tile([128, 8, QLEN], BF, tag="asb")
                f_ar = []
                for hf in range(2):
                    for dk in range(4 * hf, 4 * hf + 4):
                        fps = psP.tile([128, QLEN], DT, tag="proj")
                        for ic in range(16):
                            nc.tensor.matmul(
                                fps[:],
                                w2gs[ic // 4][:, ic % 4, 128 * dk : 128 * dk + 128],
                                hT[:, ic, :],
                                start=(ic == 0), stop=(ic == 15),
                            )
                        b2c = lnp_t[:, 4, l, dk : dk + 1]
                        if dk % 2 == 0:
                            nc.vector.tensor_scalar(fsb[:, dk, :], fps[:], b2c,
                                                    None, mybir.AluOpType.add)
                        else:
                            nc.scalar.add(fsb[:, dk, :], fps[:], b2c)
                    f_ar.append(xch_send_half(fsb, hf, 1))
                    xch_local_add(fsb, hf)
                f_cc = xch_barrier(f_ar)
                mu2_ps = psM.tile([1, QLEN], DT, tag="mm")
                e22_ps = psM.tile([1, QLEN], DT, tag="mm")
                for hf in range(2):
                    xch_recv_half(hf, 1, f_cc)
                    ln_stats(l, 1, range(4 * hf, 4 * hf + 4), mu2_ps, e22_ps)
                layer_norm(l, 1, mu2_ps, e22_ps)

            # ---- final hidden out + unembed partials ----
            nc.sync.dma_start(xout[:], xT[:])
            for vt in range(NVT):
                et = wsp.tile([128, 8, VT], BF, tag="w1g")
                nc.sync.dma_start(et[:], embT_in[vt])
                for ic in range(4):
                    lps = psM.tile([128, QLEN], DT, tag="mm")
                    for dc in range(8):
                        nc.tensor.matmul(
                            lps[:, 0:VT],
                            xT[:, dc, 128 * ic : 128 * ic + 128],
                            et[:, dc, :],
                            start=(dc == 0), stop=(dc == 7),
                        )
                    nc.vector.tensor_reduce(
                        lmax_sb[:, ic, vt : vt + 1], lps[:, 0:VT],
                        mybir.AxisListType.X, mybir.AluOpType.max,
                    )
                    negm = hdscr.tile([128, 1], DT, tag="negm")
                    nc.vector.tensor_scalar_mul(
                        negm[:], lmax_sb[:, ic, vt : vt + 1], -1.0
                    )
                    lsc = pbp.tile([128, VT], BF, tag="prb")
                    nc.scalar.activation(
                        lsc[:], lps[:, 0:VT], mybir.ActivationFunctionType.Exp,
                        bias=negm[:], accum_out=lsum_sb[:, ic, vt : vt + 1],
                    )
            nc.sync.dma_start(lmax_out[:], lmax_sb[:])
            nc.sync.dma_start(lsum_out[:], lsum_sb[:])

    nc.compile()
    return nc


def _get_nc():
    if "nc" not in _CACHE:
        _CACHE["nc"] = _build()
    return _CACHE["nc"]


def _make_pos():
    pos_seq = np.arange(KLEN - 1, -1, -1, dtype=F32)
    inv_freq = 1.0 / (10000.0 ** (np.arange(0, D_MODEL, 2, dtype=F32) / D_MODEL))
    sin_inp = np.outer(pos_seq, inv_freq).astype(F32)
    return np.concatenate([np.sin(sin_inp), np.cos(sin_inp)], -1).astype(F32)


def _chunkT(a, nch):
    """[L, D, N] -> [L, 128, nch, N]: out[l, p, c, n] = a[l, 128*c+p, n]"""
    L_, D_, N_ = a.shape
    return np.ascontiguousarray(
        a.reshape(L_, nch, 128, N_).transpose(0, 2, 1, 3)).astype(BF16)


def _colsT(a):
    """[L, N] -> [L, 128, N//128] with out[l, p, c] = a[l, 128*c+p]"""
    L_, N_ = a.shape
    return np.ascontiguousarray(
        a.reshape(L_, N_ // 128, 128).transpose(0, 2, 1)).astype(F32)


def _prep_inputs(data, memory, emb, Wq, Wkv, Wr, Wo, ffW1, ffb1, ffW2, ffb2,
                 ln1_g, ln1_b, ln2_g, ln2_b, bias_w, bias_r):
    pos = _make_pos()                                  # [KLEN, D_MODEL]
    rk = np.einsum("kd,ldn->lkn", pos, Wr.astype(F32))  # [L, KLEN, 2*NDH]
    embT = np.ascontiguousarray(emb.T).astype(BF16)    # [D_MODEL, VOCAB]
    bwf = bias_w.reshape(-1).astype(F32)
    brf = bias_r.reshape(-1).astype(F32)

    in_maps = []
    for c in range(NCORES):
        b, h = c // 2, c % 2
        nds = slice(NDH * h, NDH * h + NDH)
        dis = slice(DIH * h, DIH * h + DIH)
        # rkT[l, p, ncc, j'] = rk[l, j', nds + 128*ncc + p]
        rkTh = np.ascontiguousarray(
            rk[:, :, nds].transpose(0, 2, 1).reshape(L, 4, 128, KLEN)
            .transpose(0, 2, 1, 3)).astype(BF16)
        # memT[l, p, dc, j] = memory[l, b, j, 128*dc + p]
        memTb = _chunkT(
            np.ascontiguousarray(memory[:, b].transpose(0, 2, 1)), 8)
        x0 = emb[np.asarray(data[b])].astype(F32)      # [512, 1024]
        x0T = np.ascontiguousarray(
            x0.T.reshape(8, 128, QLEN).transpose(1, 0, 2)).astype(BF16)
        # w1 grouped: [L, 4, 128, 8, 512]; w1g[l,g,p,dc,c] = W1[l, 128dc+p, dis+512g+c]
        A = np.asarray(ffW1[:, :, dis], dtype=F32)      # [L, 1024, 2048]
        w1g = np.ascontiguousarray(
            A.reshape(L, 8, 128, 4, 512).transpose(0, 3, 2, 1, 4)).astype(BF16)
        # w2 grouped: [L, 4, 128, 4, 1024];
        # w2[l,g,p,i4,d] = W2[l, dis + 128*(4g+i4) + p, d]
        Bm = np.asarray(ffW2[:, dis, :], dtype=F32)     # [L, 2048, 1024]
        w2c = np.ascontiguousarray(
            Bm.reshape(L, 4, 4, 128, D_MODEL).transpose(0, 1, 3, 2, 4)).astype(BF16)
        embTh = embT[:, VSH * h : VSH * h + VSH]
        embT4 = np.ascontiguousarray(
            embTh.reshape(8, 128, NVT, VT).transpose(2, 1, 0, 3))
        in_maps.append({
            "x0T": x0T,
            "memT": memTb,
            "wq": _chunkT(np.asarray(Wq[:, :, nds], dtype=F32), 8),
            "wk": _chunkT(np.asarray(Wkv[:, :, nds], dtype=F32), 8),
            "wv": _chunkT(np.asarray(
                Wkv[:, :, D_MODEL + NDH * h : D_MODEL + NDH * h + NDH],
                dtype=F32), 8),
            "rkT": rkTh,
            "wo": _chunkT(np.asarray(Wo[:, nds, :], dtype=F32), 4),
            "w1": w1g,
            "w2": w2c,
            "b1": np.ascontiguousarray(
                np.asarray(ffb1[:, dis], dtype=F32)
                .reshape(L, 16, 128).transpose(0, 2, 1)),
            "b2T": _colsT(np.asarray(ffb2, dtype=F32) * 0.5),  # AR doubles it
            "g1": _colsT(np.asarray(ln1_g, dtype=F32)),
            "bg1": _colsT(np.asarray(ln1_b, dtype=F32)),
            "g2": _colsT(np.asarray(ln2_g, dtype=F32)),
            "bg2": _colsT(np.asarray(ln2_b, dtype=F32)),
            "bw": np.ascontiguousarray(bwf[nds].reshape(4, 128).T),
            "br": np.ascontiguousarray(brf[nds].reshape(4, 128).T),
            "embT": embT4,
            "ident": np.eye(128, dtype=F32).astype(BF16),
        })
    return in_maps


def _combine(results, target, emb):
    nll = np.zeros((BSZ, QLEN), dtype=np.float64)
    for b in range(BSZ):
        r0, r1 = results[2 * b], results[2 * b + 1]
        lm = np.concatenate([r0["lmax"], r1["lmax"]], axis=-1).astype(np.float64)
        ls = np.concatenate([r0["lsum"], r1["lsum"]], axis=-1).astype(np.float64)
        M = lm.max(-1)                                   # [128, 4]
        Z = (ls * np.exp(lm - M[..., None])).sum(-1)     # [128, 4]
        logZ = (M + np.log(Z)).transpose(1, 0).reshape(QLEN)  # i = 128*ic + p
        # xoutT[p, dc, i] = x[i, 128*dc+p]
        xTb = r0["xoutT"].astype(np.float64)             # [128, 8, 512]
        xf = xTb.transpose(2, 1, 0).reshape(QLEN, D_MODEL)
        et = emb[np.asarray(target[b])].astype(BF16).astype(np.float64)
        tgt = (xf * et).sum(-1)
        nll[b] = logZ - tgt
    return nll.astype(F32).reshape(-1).reshape(QLEN, BSZ)


def kernel(**inputs):
    nc = _get_nc()
    data = np.asarray(inputs["data"])
    target = np.asarray(inputs["target"])
    emb = np.asarray(inputs["emb"], dtype=F32)
    in_maps = _prep_inputs(
        data, np.asarray(inputs["memory"], dtype=F32), emb,
        np.asarray(inputs["Wq"], dtype=F32), np.asarray(inputs["Wkv"], dtype=F32),
        np.asarray(inputs["Wr"], dtype=F32), np.asarray(inputs["Wo"], dtype=F32),
        np.asarray(inputs["ffW1"], dtype=F32), np.asarray(inputs["ffb1"], dtype=F32),
        np.asarray(inputs["ffW2"], dtype=F32), np.asarray(inputs["ffb2"], dtype=F32),
        np.asarray(inputs["ln1_g"], dtype=F32), np.asarray(inputs["ln1_b"], dtype=F32),
        np.asarray(inputs["ln2_g"], dtype=F32), np.asarray(inputs["ln2_b"], dtype=F32),
        np.asarray(inputs["bias_w"], dtype=F32), np.asarray(inputs["bias_r"], dtype=F32),
    )
    res = run_bass_kernel_spmd(nc, in_maps, core_ids=list(range(NCORES)))
    return _combine(res.results, target, emb)

